# revision 4
# baseline (speedup 1.0000x reference)
"""Trainium2 Bass kernel for nn_Loss_Labels_19825569038545.

Computes -mean(log_softmax(concat([syn, ant], axis=1), axis=1)) over B=2^24
rows.

Math: per row with d = syn - ant, the row's loss contribution is
    -(lp0 + lp1) = softplus(d) + softplus(-d) = d + 2*ln(1 + e^{-d})
(no abs needed), so with u = e^{-d}, v = 1 + u:
    loss = [ sum_rows d  +  2 * sum_groups ln(prod_{i in group} v_i) ] / (2B).

Device pipeline (pure data parallel over 8 cores, B/8 = 2^21 rows/core,
laid out [128 partitions x 16384 cols] bf16):
  DVE : d = s - a                  (tensor_sub, 4x bf16 mode)
  ACT : u = Exp(-d)                (the single full-length transcendental
                                    pass, ~13.7us -> the pipeline bottleneck)
  DVE : v = u + 1                  (in place)
  DVE+Pool : 4-level pairwise product tree -> p = prod of 16 v's
             (Pool, which only supports tensor_tensor at ~0.43 eff, takes
              one half of level 1)
  PE  : ones^T @ d accumulated 512-cols-folded into PSUM -> column sums
        (32 matmuls/pass; recovers sum_rows d on an otherwise idle engine)
  DVE : copy PSUM colsums -> SBUF
  DMA : p [128, 1024] bf16 and colsum [1, 512] fp32 -> host
Host: loss = (sum(colsum) + 2*sum(ln p)) / (2B) in float64. v >= 1 so the
tree can't underflow and no clamping is needed; total rel err ~3e-5
(bf16 input quantization dominates).

Why this shape: ACT runs 1 elem/cycle/partition at 1.2 GHz regardless of
dtype, so exactly one transcendental pass per row is the floor; ln moves to
the host via the group products (no act table holds both exp and ln, and a
device ln would double ACT time). accum_out is avoided everywhere (measured
1.5x slower than the plain op). DVE tensor_tensor runs in the 4x bf16 mode
(~0.26 ns/col), stt only 2x, Pool has no stt opcode.

Raw Bass (no TileContext); per-engine program order plus explicit
semaphores (back-to-back waits are legal) sequence the 5-engine pipeline,
including the repeat>1 benchmarking variants (distinct>1 cycles separate
DRAM regions so HBM stays cold).
"""

import sys
from contextlib import ExitStack

import numpy as np
import ml_dtypes

try:
    import concourse.bass  # noqa: F401
except ImportError:
    sys.path.insert(0, "/opt/trn_rl_repo")

import concourse.bass as bass
import concourse.mybir as mybir
from concourse.bass_utils import run_bass_kernel_spmd

B = 16777216
N_CORES = 8
P = 128
WTOT = B // N_CORES // P        # 16384 cols per partition per core
NPC = 4                         # DMA pieces / sub ops per pass
PW = WTOT // NPC                # 4096
G = 16                          # product-group size
NP_OUT = WTOT // G              # 1024 p-values per partition
CS = 512                        # PSUM colsum width (one bank)

FP32 = mybir.dt.float32
BF16 = mybir.dt.bfloat16

# DVE ops per pass, in stream order:
#   1-4: sub x4 | 5: v0 | 6: v1 | 7: L1a | 8: L2 | 9: L3 | 10: L4 | 11: cs
N_DVE = 11
N_ACT = 2                       # two Exp instrs (8192 wide)
N_POOL = 1                      # L1b
N_PE = WTOT // CS               # 32 matmuls

_nc_cache = {}


def _build_nc(repeat=1, distinct=1):
    key = (repeat, distinct)
    if key in _nc_cache:
        return _nc_cache[key]
    nc = bass.Bass()
    sa = nc.dram_tensor("sa", [distinct, P * 2 * WTOT], BF16,
                        kind="ExternalInput")
    out = nc.dram_tensor("out", [P, NP_OUT], BF16, kind="ExternalOutput")
    cs_out = nc.dram_tensor("cs_out", [1, CS], FP32, kind="ExternalOutput")

    H = WTOT // 2               # 8192
    Q = WTOT // 4               # 4096
    E = WTOT // 8               # 2048

    with ExitStack() as ctx:
        sb = ctx.enter_context
        sa_b = sb(nc.sbuf_tensor("sa_b", [P, 2 * WTOT], BF16))
        d_b = sb(nc.sbuf_tensor("d_b", [P, WTOT], BF16))
        u_b = sb(nc.sbuf_tensor("u_b", [P, WTOT], BF16))
        one_b = sb(nc.sbuf_tensor("one_b", [P, H], BF16))
        tb = sb(nc.sbuf_tensor("tb", [P, H], BF16))
        p_b = sb(nc.sbuf_tensor("p_b", [P, NP_OUT], BF16))
        cs_b = sb(nc.sbuf_tensor("cs_b", [1, CS], FP32))
        ones_w = sb(nc.sbuf_tensor("ones_w", [P, 1], BF16))
        cs_ps = sb(nc.psum_tensor("cs_ps", [1, CS], FP32))
        ld = sb(nc.semaphore("ld"))
        dve_p = sb(nc.semaphore("dve_p"))
        act_p = sb(nc.semaphore("act_p"))
        pool_p = sb(nc.semaphore("pool_p"))
        pe_p = sb(nc.semaphore("pe_p"))
        st = sb(nc.semaphore("st"))
        block = ctx.enter_context(nc.Block())

        @block.sync
        def _(sync):
            for r in range(repeat):
                for p in range(NPC):
                    if r > 0:
                        # sa piece p consumed by sub p of pass r-1
                        sync.wait_ge(dve_p, (r - 1) * N_DVE + p + 1)
                    base = P * 2 * PW * p
                    sync.dma_start(
                        out=sa_b[:, 2 * PW * p: 2 * PW * (p + 1)],
                        in_=sa[r % distinct, base: base + P * 2 * PW]
                        .rearrange("(p c) -> p c", p=P),
                    ).then_inc(ld, 16)
                sync.wait_ge(dve_p, r * N_DVE + 10)      # L4 done
                sync.dma_start(out=out[:], in_=p_b[:]).then_inc(st, 16)
                sync.wait_ge(dve_p, (r + 1) * N_DVE)     # cs copy done
                sync.dma_start(out=cs_out[:], in_=cs_b[:]).then_inc(st, 16)
            sync.wait_ge(st, 32 * repeat)

        @block.scalar
        def _(act):
            for r in range(repeat):
                for h in range(2):
                    # needs sub 2h, 2h+1 of this pass; u_b half h free once
                    # L1a (dve, subsumed) and L1b (pool) of pass r-1 are done
                    if h == 0 and r > 0:
                        act.wait_ge(pool_p, r)
                    act.wait_ge(dve_p, r * N_DVE + 2 * (h + 1))
                    act.activation(
                        out=u_b[:, h * H:(h + 1) * H],
                        in_=d_b[:, h * H:(h + 1) * H],
                        func=mybir.ActivationFunctionType.Exp,
                        scale=-1.0,
                    ).then_inc(act_p, 1)

        @block.gpsimd
        def _(pool):
            for r in range(repeat):
                # L1b: tb[Q:H] = v[Q:H] * v[H+Q:WTOT]; v1 is dve op 6
                pool.wait_ge(dve_p, r * N_DVE + 6)
                pool.tensor_mul(
                    out=tb[:, Q:H], in0=u_b[:, Q:H], in1=u_b[:, H + Q:WTOT],
                ).then_inc(pool_p, 1)

        @block.tensor
        def _(pe):
            for r in range(repeat):
                for k in range(N_PE):
                    p = (k * CS) // PW
                    pe.wait_ge(dve_p, r * N_DVE + p + 1)
                    pe.matmul(
                        out=cs_ps[:],
                        lhsT=ones_w[:],
                        rhs=d_b[:, k * CS:(k + 1) * CS],
                        start=(k == 0),
                        stop=(k == N_PE - 1),
                    ).then_inc(pe_p, 1)

        @block.vector
        def _(dve):
            dve.memset(one_b[:], 1.0)
            dve.memset(ones_w[:], 1.0)
            for r in range(repeat):
                for p in range(NPC):
                    dve.wait_ge(ld, 16 * (r * NPC + p + 1))
                    if r > 0 and p == 0:
                        # d_b readers in pass r-1: Exp (act) and PE matmuls
                        dve.wait_ge(act_p, 2 * r)
                        dve.wait_ge(pe_p, N_PE * r)
                    dve.tensor_sub(
                        out=d_b[:, PW * p: PW * (p + 1)],
                        in0=sa_b[:, 2 * PW * p: 2 * PW * p + PW],
                        in1=sa_b[:, 2 * PW * p + PW: 2 * PW * (p + 1)],
                    ).then_inc(dve_p, 1)
                # v = u + 1 in place, halves (tensor_tensor add with a ones
                # tile keeps the 4x mode; tensor_scalar would be 2x)
                for h in range(2):
                    dve.wait_ge(act_p, r * N_ACT + h + 1)
                    dve.tensor_add(
                        out=u_b[:, h * H:(h + 1) * H],
                        in0=u_b[:, h * H:(h + 1) * H],
                        in1=one_b[:],
                    ).then_inc(dve_p, 1)
                # L1a: tb[0:Q] = v[0:Q] * v[H:H+Q]; tb[0:Q] free once L3 of
                # pass r-1 is done (own stream)
                dve.tensor_mul(
                    out=tb[:, 0:Q], in0=u_b[:, 0:Q], in1=u_b[:, H:H + Q],
                ).then_inc(dve_p, 1)
                # L2 pairs the DVE and Pool L1 halves
                dve.wait_ge(pool_p, r + 1)
                dve.tensor_mul(
                    out=tb[:, 0:Q], in0=tb[:, 0:Q], in1=tb[:, Q:H],
                ).then_inc(dve_p, 1)
                dve.tensor_mul(
                    out=tb[:, 0:E], in0=tb[:, 0:E], in1=tb[:, E:Q],
                ).then_inc(dve_p, 1)
                if r > 0:
                    dve.wait_ge(st, 32 * r)   # p_b, cs_b consumed by DMA r-1
                dve.tensor_mul(
                    out=p_b[:], in0=tb[:, 0:NP_OUT], in1=tb[:, NP_OUT:E],
                ).then_inc(dve_p, 1)
                # colsum PSUM -> SBUF once the accumulation group closed
                dve.wait_ge(pe_p, N_PE * (r + 1))
                dve.tensor_copy(out=cs_b[:], in_=cs_ps[:]).then_inc(dve_p, 1)

    _nc_cache[key] = nc
    return nc


def _pack_sa(synonymy_score, antonymy_score):
    """Per core: [1, P*2*WTOT] bf16; NPC contiguous per-piece slabs, each
    partition row holding the s piece then the a piece."""
    s = np.asarray(synonymy_score, dtype=np.float32).reshape(
        N_CORES, P, NPC, PW)
    a = np.asarray(antonymy_score, dtype=np.float32).reshape(
        N_CORES, P, NPC, PW)
    sa = np.stack([s, a], axis=3)               # [C, P, NPC, 2, PW]
    sa = sa.transpose(0, 2, 1, 3, 4)            # [C, NPC, P, 2, PW]
    return np.ascontiguousarray(sa).reshape(
        N_CORES, 1, P * 2 * WTOT).astype(ml_dtypes.bfloat16)


def _run(synonymy_score, antonymy_score, **spmd_kwargs):
    nc = _build_nc()
    sa = _pack_sa(synonymy_score, antonymy_score)
    in_maps = [{"sa": sa[c]} for c in range(N_CORES)]
    r = run_bass_kernel_spmd(nc, in_maps, list(range(N_CORES)), **spmd_kwargs)
    tot = np.float64(0.0)
    for c in range(N_CORES):
        p = r.results[c]["out"].astype(np.float64)
        tot += 2.0 * np.log(p).sum()
        tot += r.results[c]["cs_out"].astype(np.float64).sum()
    value = np.asarray(tot / (2.0 * B), dtype=np.float32)
    return value, r


def kernel(S1_out, synonymy_score, antonymy_score):
    return _run(synonymy_score, antonymy_score)[0]


# revision 7
# speedup vs baseline: 1.3877x; 1.3877x over previous
"""Trainium2 Bass kernel for nn_Loss_Labels_19825569038545.

Computes -mean(log_softmax(concat([syn, ant], axis=1), axis=1)) over B=2^24
rows.

Math: per row with d = syn - ant, the row's loss contribution is
    -(lp0 + lp1) = softplus(d) + softplus(-d) = d + 2*ln(1 + e^{-d})
(no abs needed), so with u = e^{-d}, v = 1 + u:
    loss = [ sum_rows d  +  2 * sum_groups ln(prod_{i in group} v_i) ] / (2B).

Device pipeline (pure data parallel over 8 cores, B/8 = 2^21 rows/core,
laid out [128 partitions x 16384 cols] bf16):
  DVE : d = s - a                   (tensor_sub, 4x bf16 mode)
  ACT : u = Exp(-d), 4 quarter-instrs (the single full-length transcendental
        pass, ~14us -> the pipeline bottleneck; ACT is 1 elem/cycle/partition
        at 1.2 GHz for any dtype, so one pass per row is the floor)
  DVE : v = u + 1 in place per quarter (tensor_tensor add with a ones tile:
        tensor_scalar only runs at 2x, tensor_tensor at 4x)
  DVE : 4-level pairwise product tree, fully in place in the u buffer,
        tail kept in the upper half -> p = prod of 16 v's
  PE  : ones^T @ d accumulated 512-col-folded into one PSUM bank -> column
        sums (32 matmuls/pass; recovers sum_rows d on an otherwise idle
        engine; v >= 1 makes the tree underflow-free, and this sum makes the
        abs pass unnecessary)
  DMA : p [128, 1024] bf16, colsum [1, 512] fp32 -> host
Host: loss = (sum(colsum) + 2*sum(ln p)) / (2B) in float64.
Total rel err ~3e-5 (bf16 input quantization dominates).

Scheduling: d is double-buffered by pass parity so the subs of pass r never
wait on Exp/PE of pass r-1; the exp quarters' only waits are on this pass's
subs (all previous-pass tree reads of u are subsumed by those counts); the
tree tail lives in the upper half of u so exp quarter k of pass r+1 only
conflicts with tree ops that finished a full pass earlier. Output DMAs of
pass r are issued after the input DMAs of pass r+1 so input prefetch is
never blocked behind compute waits. The Ln stays on the host: no act table
holds both exp and ln, and a device ln would double ACT time. accum_out is
avoided everywhere (measured 1.5x slower than the plain op); Pool is unused
(no stt opcode, tensor ops at 0.43 eff would sit on the critical path).

Raw Bass (no TileContext); per-engine program order plus explicit
semaphores (back-to-back waits are legal) sequence the pipeline, including
the repeat>1 benchmarking variants (distinct>1 cycles separate DRAM
regions so HBM stays cold).
"""

import sys
from contextlib import ExitStack

import numpy as np
import ml_dtypes

try:
    import concourse.bass  # noqa: F401
except ImportError:
    sys.path.insert(0, "/opt/trn_rl_repo")

import concourse.bass as bass
import concourse.mybir as mybir
from concourse.bass_utils import run_bass_kernel_spmd

B = 16777216
N_CORES = 8
P = 128
WTOT = B // N_CORES // P        # 16384 cols per partition per core
NPC = 4                         # DMA pieces / sub ops / exp quarters per pass
PW = WTOT // NPC                # 4096
G = 16                          # product-group size
NP_OUT = WTOT // G              # 1024 p-values per partition
CS = 512                        # PSUM colsum width (one bank)

FP32 = mybir.dt.float32
BF16 = mybir.dt.bfloat16

# DVE ops per pass, in stream order:
#  1-4: sub q0-q3 | 5: v_q0 | 6: v_q1 | 7: v_q2 | 8: L1a | 9: v_q3
#  | 10: L1b | 11: L2 | 12: L3 | 13: L4 | 14: cs copy (PSUM -> SBUF)
N_DVE = 14
N_ACT = 4                       # four Exp quarter-instrs
N_PE = WTOT // CS               # 32 matmuls

H = WTOT // 2                   # 8192
Q = WTOT // 4                   # 4096
E = WTOT // 8                   # 2048

_nc_cache = {}


def _build_nc(repeat=1, distinct=1):
    key = (repeat, distinct)
    if key in _nc_cache:
        return _nc_cache[key]
    nc = bass.Bass()
    sa = nc.dram_tensor("sa", [distinct, P * 2 * WTOT], BF16,
                        kind="ExternalInput")
    out = nc.dram_tensor("out", [P, NP_OUT], BF16, kind="ExternalOutput")
    cs_out = nc.dram_tensor("cs_out", [1, CS], FP32, kind="ExternalOutput")

    with ExitStack() as ctx:
        sb = ctx.enter_context
        sa_b = sb(nc.sbuf_tensor("sa_b", [P, 2 * WTOT], BF16))
        d_bufs = [sb(nc.sbuf_tensor(f"d{i}", [P, WTOT], BF16))
                  for i in range(2)]
        u_b = sb(nc.sbuf_tensor("u_b", [P, WTOT], BF16))
        one_b = sb(nc.sbuf_tensor("one_b", [P, PW], BF16))
        p_b = sb(nc.sbuf_tensor("p_b", [P, NP_OUT], BF16))
        ones_w = sb(nc.sbuf_tensor("ones_w", [P, 1], BF16))
        cs_b = sb(nc.sbuf_tensor("cs_b", [1, CS], FP32))
        cs_ps = sb(nc.psum_tensor("cs_ps", [1, CS], FP32))
        ld = sb(nc.semaphore("ld"))
        dve_p = sb(nc.semaphore("dve_p"))
        act_p = sb(nc.semaphore("act_p"))
        pe_p = sb(nc.semaphore("pe_p"))
        pst = sb(nc.semaphore("pst"))
        cst = sb(nc.semaphore("cst"))
        block = ctx.enter_context(nc.Block())

        @block.sync
        def _(sync):
            for r in range(repeat):
                for p in range(NPC):
                    if r > 0:
                        # sa piece p consumed by sub p of pass r-1
                        sync.wait_ge(dve_p, (r - 1) * N_DVE + p + 1)
                    base = P * 2 * PW * p
                    sync.dma_start(
                        out=sa_b[:, 2 * PW * p: 2 * PW * (p + 1)],
                        in_=sa[r % distinct, base: base + P * 2 * PW]
                        .rearrange("(p c) -> p c", p=P),
                    ).then_inc(ld, 16)
                if r > 0:
                    # emit the previous pass's outputs after this pass's
                    # input DMAs so prefetch is never blocked
                    sync.wait_ge(dve_p, (r - 1) * N_DVE + 13)     # L4
                    sync.dma_start(out=out[:], in_=p_b[:]).then_inc(pst, 16)
                    sync.wait_ge(dve_p, (r - 1) * N_DVE + 14)     # cs copy
                    sync.dma_start(out=cs_out[:], in_=cs_b[:]) \
                        .then_inc(cst, 16)
            sync.wait_ge(dve_p, (repeat - 1) * N_DVE + 13)
            sync.dma_start(out=out[:], in_=p_b[:]).then_inc(pst, 16)
            sync.wait_ge(dve_p, repeat * N_DVE)
            sync.dma_start(out=cs_out[:], in_=cs_b[:]).then_inc(cst, 16)
            sync.wait_ge(pst, 16 * repeat)
            sync.wait_ge(cst, 16 * repeat)

        @block.scalar
        def _(act):
            for r in range(repeat):
                d_b = d_bufs[r % 2]
                for q in range(NPC):
                    # sub q of this pass; all previous-pass tree reads of
                    # this u quarter are subsumed by that dve count
                    act.wait_ge(dve_p, r * N_DVE + q + 1)
                    act.activation(
                        out=u_b[:, q * PW:(q + 1) * PW],
                        in_=d_b[:, q * PW:(q + 1) * PW],
                        func=mybir.ActivationFunctionType.Exp,
                        scale=-1.0,
                    ).then_inc(act_p, 1)

        @block.tensor
        def _(pe):
            for r in range(repeat):
                d_b = d_bufs[r % 2]
                for k in range(N_PE):
                    p = (k * CS) // PW
                    # (PSUM freed by the cs copy, dve op 14 of pass r-1,
                    #  which the piece wait below subsumes)
                    pe.wait_ge(dve_p, r * N_DVE + p + 1)
                    pe.matmul(
                        out=cs_ps[:],
                        lhsT=ones_w[:],
                        rhs=d_b[:, k * CS:(k + 1) * CS],
                        start=(k == 0),
                        stop=(k == N_PE - 1),
                    ).then_inc(pe_p, 1)

        @block.vector
        def _(dve):
            dve.memset(one_b[:], 1.0)
            dve.memset(ones_w[:], 1.0)
            for r in range(repeat):
                d_b = d_bufs[r % 2]
                for p in range(NPC):
                    dve.wait_ge(ld, 16 * (r * NPC + p + 1))
                    if r > 1 and p == 0:
                        # d buffer r%2 was last read by Exp/PE of pass r-2
                        dve.wait_ge(act_p, NPC * (r - 1))
                        dve.wait_ge(pe_p, N_PE * (r - 1))
                    dve.tensor_sub(
                        out=d_b[:, PW * p: PW * (p + 1)],
                        in0=sa_b[:, 2 * PW * p: 2 * PW * p + PW],
                        in1=sa_b[:, 2 * PW * p + PW: 2 * PW * (p + 1)],
                    ).then_inc(dve_p, 1)

                def v_q(q):
                    dve.wait_ge(act_p, r * N_ACT + q + 1)
                    dve.tensor_add(
                        out=u_b[:, q * PW:(q + 1) * PW],
                        in0=u_b[:, q * PW:(q + 1) * PW],
                        in1=one_b[:],
                    ).then_inc(dve_p, 1)

                v_q(0)
                v_q(1)
                v_q(2)
                # L1a: u[2Q:3Q] = v[0:Q] * v[2Q:3Q]  (frees u quarter 0)
                dve.tensor_mul(
                    out=u_b[:, 2 * Q:3 * Q], in0=u_b[:, 0:Q],
                    in1=u_b[:, 2 * Q:3 * Q],
                ).then_inc(dve_p, 1)
                v_q(3)
                # L1b: u[3Q:4Q] = v[Q:2Q] * v[3Q:4Q]  (frees u quarter 1)
                dve.tensor_mul(
                    out=u_b[:, 3 * Q:WTOT], in0=u_b[:, Q:2 * Q],
                    in1=u_b[:, 3 * Q:WTOT],
                ).then_inc(dve_p, 1)
                # tail lives in u[2Q:): L2, L3, then L4 -> p_b
                dve.tensor_mul(
                    out=u_b[:, 2 * Q:3 * Q], in0=u_b[:, 2 * Q:3 * Q],
                    in1=u_b[:, 3 * Q:WTOT],
                ).then_inc(dve_p, 1)
                dve.tensor_mul(
                    out=u_b[:, 2 * Q:2 * Q + E], in0=u_b[:, 2 * Q:2 * Q + E],
                    in1=u_b[:, 2 * Q + E:3 * Q],
                ).then_inc(dve_p, 1)
                if r > 0:
                    dve.wait_ge(pst, 16 * r)   # p_b consumed by out-DMA r-1
                dve.tensor_mul(
                    out=p_b[:], in0=u_b[:, 2 * Q:2 * Q + NP_OUT],
                    in1=u_b[:, 2 * Q + NP_OUT:2 * Q + E],
                ).then_inc(dve_p, 1)
                dve.wait_ge(pe_p, N_PE * (r + 1))
                if r > 0:
                    dve.wait_ge(cst, 16 * r)  # cs_b consumed by out-DMA r-1
                dve.tensor_copy(out=cs_b[:], in_=cs_ps[:]).then_inc(dve_p, 1)

    _nc_cache[key] = nc
    return nc


def _pack_sa(synonymy_score, antonymy_score):
    """Per core: [1, P*2*WTOT] bf16; NPC contiguous per-piece slabs, each
    partition row holding the s piece then the a piece."""
    s = np.asarray(synonymy_score, dtype=np.float32).reshape(
        N_CORES, P, NPC, PW)
    a = np.asarray(antonymy_score, dtype=np.float32).reshape(
        N_CORES, P, NPC, PW)
    sa = np.stack([s, a], axis=3)               # [C, P, NPC, 2, PW]
    sa = sa.transpose(0, 2, 1, 3, 4)            # [C, NPC, P, 2, PW]
    return np.ascontiguousarray(sa).reshape(
        N_CORES, 1, P * 2 * WTOT).astype(ml_dtypes.bfloat16)


def _run(synonymy_score, antonymy_score, **spmd_kwargs):
    nc = _build_nc()
    sa = _pack_sa(synonymy_score, antonymy_score)
    in_maps = [{"sa": sa[c]} for c in range(N_CORES)]
    r = run_bass_kernel_spmd(nc, in_maps, list(range(N_CORES)), **spmd_kwargs)
    tot = np.float64(0.0)
    for c in range(N_CORES):
        p = r.results[c]["out"].astype(np.float64)
        tot += 2.0 * np.log(p).sum()
        tot += r.results[c]["cs_out"].astype(np.float64).sum()
    value = np.asarray(tot / (2.0 * B), dtype=np.float32)
    return value, r


def kernel(S1_out, synonymy_score, antonymy_score):
    return _run(synonymy_score, antonymy_score)[0]
